# revision 42
# baseline (speedup 1.0000x reference)
"""AfmoeMoE Trainium2 kernel — expert-parallel across 8 NeuronCores.

Strategy (per sharding hint): expert-parallel. The host computes the router
(cheap: [T,E] logits + grouped top-k) to build the dispatch plan, gathers each
expert's tokens into fixed-capacity slots, and ships each core 4 experts'
weights + its gathered tokens. The device runs the heavy expert MLPs
(silu(x@wg)*(x@wu) @ wd) on gathered tokens, plus 1/8 of the shared expert
(T/4 token block x I/2 intermediate half). The host then scatter-adds the
weighted expert outputs back (the "all-to-all combine") and sums the shared
partials.

Numerics: routed gate/up weights ship as float8_e3m4 scaled by 128 (max |w|
* 128 = 13.9 < 15.5 so no clipping); the matmul runs fp8 lhsT x fp16 rhs
into fp32 PSUM, and the silu activation applies the 1/128 input scale. The
128x on the up path rides through mid (fp16, well within range) and the down
matmul, and is divided out on the host during the combine. ALL routed w_down
are also e3m4 (scale 32); the shared expert stays fp16 (e3m4 there measured
2.1-2.3e-2, over budget). End-to-end rel err 1.753e-2 vs the 2e-2 budget
(measured on HW; matches the host-side emulation to 4 digits). fp8e4m3
DoubleRow (2x PE rate) was measured numerically infeasible: it requires BOTH
operands in e4m3, and quantizing x/mid costs 4-6e-2 end-to-end.

DMA: everything is packed host-side into the exact SBUF layout so each
logical load is ONE contiguous >=0.25MB dma_start per OPERAND (wg / wu / wd
per slot, x gathered per slot — deps are tile-granular, so per-operand tiles
let each matmul group release on exactly its own transfer), issued up front
in consumption order on the sync (HWDGE) queue. Completion sems fire in ring
order ~2-3us after a transfer's last byte (receipt round-trip), pacing
~0.7-1us apart — so the opening slot's wg additionally splits into k-half
tiles and the PE warmup is sized to end right at the first sems (~11.3us).
A second DGE ring (scalar or gpsimd) measurably HURTS the opening: it either
competes for SDMA engines or sits behind ACT_TABLE_LOAD. Outputs leave via
SWDGE (gpsimd) so their compute-gated waits never head-of-line-block the
input stream; the final slot's outputs use the by-then-idle sync ring.

Compute: gate/up matmuls keep features on partitions (rhs = gathered tokens,
C-row instructions). The down-proj swaps orientation where token padding
allows (lhsT = mid block, tokens become out partitions, 512-row matmuls into
a full PSUM bank) which quarters the instruction count; slots where C is far
from a 128 multiple keep the h-tile orientation. PSUM rings: psg 4 / psu 2 /
pso 2 banks (psg 4 lets the opening slot hold all four gate groups open on
the first k-half while the second is still in flight); slot order tuned so
the first slot needs the least data and the last slot has the shortest
post-arrival tail. Slot capacities are padded to 8 (data-derived per call;
kernel compiled per capacity tuple). An 18-matmul cold warmup (~3.8us)
doubles as the HAM clock-gate trip so every real matmul runs at 2.4GHz.

Measured on this problem/seed: 50.3us HW exec (baseline session: 73.0us).
The body is on the roofline ridge: the input stream (~9.3MB/core) is ~99%
DMA-busy for 12-33us while the PE is ~99% busy 11-45us at the bf16 streaming
rate; ~13us is fixed framework preamble/epilogue (sem init + per-sem reset),
measured on an empty kernel. Virtual splitting of hot experts (5 slots,
-2.7us PE) was tried and net-regressed: +1.5MB/core weight traffic costs
more than the PE it saves.
"""
import os
import numpy as np
import ml_dtypes

import concourse.bacc as bacc
import concourse.mybir as mybir
from concourse.tile import TileContext
from concourse import bass_utils

T, H, E, IE = 1024, 1024, 32, 512
N_GROUP, TOPK_GROUP, TOP_K = 4, 2, 4
ROUTE_SCALE = 2.5
NCORES = 8
EPC = E // NCORES          # experts per core
IS = 512                   # shared-expert intermediate
ISH = IS // 2              # shared intermediate half per core
SBLK = T // 4              # shared token-block size (256)
KT = H // 128              # k tiles over H
HT = H // 128              # output h tiles
F8 = mybir.dt.float8e3
F16 = mybir.dt.float16
F32 = mybir.dt.float32
E3M4 = ml_dtypes.float8_e3m4
WSC = 128.0                # fp8 gate/up weight scale (undone in act + host)
DSC = 32.0                 # fp8 down weight scale (all routed slots)

_nc_cache = {}
last_exec_time_ns = None


def _swap_down(C):
    """Swapped down-proj (tokens as out partitions, 512-row matmuls) streams
    B*4096 PE cycles regardless of block occupancy; the h-tile orientation
    streams 32*C. Swap only when it is not more PE cycles (it has 4x fewer
    instructions, so prefer it at equality)."""
    return 32 * C >= ((C + 127) // 128) * 4096


def _build(caps):
    """Build + compile the per-core kernel.

    caps: per-slot token capacities, slots 0..EPC-1 routed (intermediate IE,
    gate/up in fp8e3), slot EPC = shared (intermediate ISH, all fp16).
    """
    nc = bacc.Bacc("TRN2", target_bir_lowering=False, debug=False, num_devices=NCORES)

    EPCV = len(caps) - 1               # routed slots (may exceed EPC with
    #                                    virtual splitting of hot experts)
    order = [EPCV - 2] + list(range(EPCV - 2)) + [EPCV, EPCV - 1]

    xg_ds = [nc.dram_tensor(f"xg{s}", [128, KT, caps[s]], F16, kind="ExternalInput")
             for s in range(EPCV + 1)]
    wg_ds = [nc.dram_tensor(f"wg{s}", [128, KT, IE], F8, kind="ExternalInput")
             for s in range(EPCV)]
    wu_ds = [nc.dram_tensor(f"wu{s}", [128, KT, IE], F8, kind="ExternalInput")
             for s in range(EPCV)]
    wd_ds = [nc.dram_tensor(f"wd{s}", [128, IE // 128, H], F8,
                            kind="ExternalInput")
             for s in range(EPCV)]
    swg_d = nc.dram_tensor("swg", [128, KT * ISH], F16, kind="ExternalInput")
    swu_d = nc.dram_tensor("swu", [128, KT * ISH], F16, kind="ExternalInput")
    swd_d = nc.dram_tensor("swd", [128, (ISH // 128) * H], F16, kind="ExternalInput")
    yg_ds = [nc.dram_tensor(
                 f"yg{s}",
                 [128, (caps[s] + 127) // 128, H] if _swap_down(caps[s])
                 else [128, HT, caps[s]],
                 F16, kind="ExternalOutput")
             for s in range(EPCV + 1)]

    with TileContext(nc) as tc:
        with tc.tile_pool(name="xp", bufs=1) as xp, \
             tc.tile_pool(name="wp", bufs=1) as wp, \
             tc.tile_pool(name="mp", bufs=4) as mp, \
             tc.tile_pool(name="op", bufs=3) as op, \
             tc.tile_pool(name="cn", bufs=1) as cn, \
             tc.tile_pool(name="ps", bufs=3, space="PSUM") as ps:

            # PE warmup: cold dummy matmuls from ~6.6us (gpsimd memset is
            # ready ~1us before the vector engine) until the first slot's
            # opening transfer completion sems fire (~11.4us: issue + first
            # byte + bytes + ~2us receipt). >=3.4us of sustained PE busy
            # trips the HAM SHORT window so the real matmuls run at 2.4GHz.
            wtile = cn.tile([128, 384], F16)
            nc.gpsimd.memset(wtile, 0.0)
            pswarm = ps.tile([128, 256], F32, tag="pso", bufs=2)
            for r in range(18):
                nc.tensor.matmul(pswarm, lhsT=wtile[:, :128], rhs=wtile[:, 128:],
                                 start=(r == 0), stop=(r == 18 - 1))

            # ---- issue every input DMA up front, in consumption order ----
            # slot order tuned on the timeline sim: the second-lightest
            # routed slot leads (small fp8 first transfer), the lightest
            # trails (smallest post-arrival tail).
            # Dependencies are TILE-granular and each transfer costs
            # ~0.65us of serialized ring overhead (descriptor-gen at issue,
            # completion receipt before the sem). So: the first two slots
            # get per-operand tiles in consumption order (wg further split
            # into k-halves for the opening), later slots ship wg|wu|wd as
            # ONE 1.5MB fp8 blob. Everything rides the sync HWDGE ring: a
            # second ring competing for SDMA engines during the opening
            # delays the critical first transfers (measured), and the
            # scalar ring is poisoned by ACT_TABLE_LOAD at its queue head.
            xg_sbs = {}
            wg_sbs = {}
            wu_sbs = {}
            wd_sbs = {}
            KH = KT // 2
            s0 = order[0]
            for si, s in enumerate(order):
                if s == EPCV:
                    xg_sbs[s] = xp.tile([128, KT, caps[s]], F16,
                                        tag=f"xg{s}", name=f"xg{s}")
                    nc.sync.dma_start(xg_sbs[s], xg_ds[s].ap())
                    swg_sb = wp.tile([128, KT * ISH], F16, tag="swg")
                    nc.sync.dma_start(swg_sb, swg_d.ap())
                    swu_sb = wp.tile([128, KT * ISH], F16, tag="swu")
                    nc.sync.dma_start(swu_sb, swu_d.ap())
                    swd_sb = wp.tile([128, (ISH // 128) * H], F16, tag="swd")
                    nc.sync.dma_start(swd_sb, swd_d.ap())
                    continue
                xg_sbs[s] = xp.tile([128, KT, caps[s]], F16,
                                    tag=f"xg{s}", name=f"xg{s}")
                wu_sbs[s] = wp.tile([128, KT, IE], F8, tag=f"wu{s}",
                                    name=f"wu{s}")
                wd_sbs[s] = wp.tile([128, IE // 128, H], F8,
                                    tag=f"wd{s}", name=f"wdt{s}")
                nc.sync.dma_start(xg_sbs[s], xg_ds[s].ap())
                if si == 0:
                    # wg k-half tiles: the opening matmul groups
                    # release on xg + wgA alone
                    wgA = wp.tile([128, KH, IE], F8, tag="wgA", name="wgA")
                    wgB = wp.tile([128, KH, IE], F8, tag="wgB", name="wgB")
                    wg_sbs[s] = (wgA, wgB)
                    nc.sync.dma_start(wgA, wg_ds[s][:, :KH])
                    nc.sync.dma_start(wgB, wg_ds[s][:, KH:])
                else:
                    wg_sbs[s] = wp.tile([128, KT, IE], F8, tag=f"wg{s}",
                                        name=f"wg{s}")
                    nc.sync.dma_start(wg_sbs[s], wg_ds[s].ap())
                nc.sync.dma_start(wu_sbs[s], wu_ds[s].ap())
                nc.sync.dma_start(wd_sbs[s], wd_ds[s].ap())

            # ---- compute, slot by slot ----
            for s in order:
                C = caps[s]
                shared = s == EPCV
                I_s = ISH if shared else IE
                it = I_s // 128
                if shared:
                    xg_sb = xg_sbs[s]

                    def x_r(k):
                        return xg_sb[:, k, :]

                    def g_l(k, i):
                        return swg_sb[:, k * ISH + i * 128:k * ISH + (i + 1) * 128]

                    def u_l(k, i):
                        return swu_sb[:, k * ISH + i * 128:k * ISH + (i + 1) * 128]

                    def d_r(k, hh):
                        return swd_sb[:, k * H + hh * 512:k * H + hh * 512 + 512]

                    def d_c(k, h):
                        return swd_sb[:, k * H + h * 128:k * H + (h + 1) * 128]
                    act_scale = 1.0
                else:
                    xg_sb = xg_sbs[s]
                    wu_sb, wd_sb = wu_sbs[s], wd_sbs[s]

                    def x_r(k, x=xg_sb):
                        return x[:, k, :]

                    if s == s0:
                        wgAB = wg_sbs[s]

                        def g_l(k, i, w=wgAB):
                            return w[k // KH][:, k % KH, i * 128:(i + 1) * 128]
                    else:
                        wg_sb = wg_sbs[s]

                        def g_l(k, i, w=wg_sb):
                            return w[:, k, i * 128:(i + 1) * 128]

                    def u_l(k, i, w=wu_sb):
                        return w[:, k, i * 128:(i + 1) * 128]

                    def d_r(k, hh, w=wd_sb):
                        return w[:, k, hh * 512:hh * 512 + 512]

                    def d_c(k, h, w=wd_sb):
                        return w[:, k, h * 128:(h + 1) * 128]
                    act_scale = 1.0 / WSC

                def emit_gu(i):
                    ps_g = ps.tile([128, C], F32, tag="psg", bufs=4, name="ps_g")
                    for k in range(KT):
                        nc.tensor.matmul(ps_g, lhsT=g_l(k, i), rhs=x_r(k),
                                         start=(k == 0), stop=(k == KT - 1))
                    ps_u = ps.tile([128, C], F32, tag="psu", bufs=2, name="ps_u")
                    for k in range(KT):
                        nc.tensor.matmul(ps_u, lhsT=u_l(k, i), rhs=x_r(k),
                                         start=(k == 0), stop=(k == KT - 1))
                    sil = mp.tile([128, C], F16, tag=f"sil{i}", name="sil")
                    nc.scalar.activation(sil, ps_g, mybir.ActivationFunctionType.Silu,
                                         scale=act_scale)
                    mid = mp.tile([128, C], F16, tag=f"mid{i}", name="mid")
                    nc.vector.tensor_mul(mid, sil, ps_u)
                    mids.append(mid)

                mids = []
                if s == s0:
                    # opening slot: open all 4 gate groups on the A k-halves
                    # so PE has 16 matmuls of work as soon as the first two
                    # transfers land, then close them on the B halves as
                    # those arrive, then run the u groups (wu lands later)
                    ps_gs = []
                    for i in range(it):
                        ps_g = ps.tile([128, C], F32, tag="psg", bufs=4,
                                       name="ps_g")
                        for k in range(KH):
                            nc.tensor.matmul(ps_g, lhsT=g_l(k, i), rhs=x_r(k),
                                             start=(k == 0), stop=False)
                        ps_gs.append(ps_g)
                    sils = []
                    for i in range(it):
                        for k in range(KH, KT):
                            nc.tensor.matmul(ps_gs[i], lhsT=g_l(k, i), rhs=x_r(k),
                                             start=False, stop=(k == KT - 1))
                        sil = mp.tile([128, C], F16, tag=f"sil{i}", name="sil")
                        nc.scalar.activation(sil, ps_gs[i],
                                             mybir.ActivationFunctionType.Silu,
                                             scale=act_scale)
                        sils.append(sil)
                    for i in range(it):
                        ps_u = ps.tile([128, C], F32, tag="psu", bufs=2,
                                       name="ps_u")
                        for k in range(KT):
                            nc.tensor.matmul(ps_u, lhsT=u_l(k, i), rhs=x_r(k),
                                             start=(k == 0), stop=(k == KT - 1))
                        mid = mp.tile([128, C], F16, tag=f"mid{i}", name="mid")
                        nc.vector.tensor_mul(mid, sils[i], ps_u)
                        mids.append(mid)
                else:
                    for i in range(it):
                        emit_gu(i)

                # down-proj: swapped (lhsT = mid block, tokens as out
                # partitions, 512-row matmuls) when token-block padding is
                # small; else classic h-tile orientation. Outputs leave via
                # SWDGE (gpsimd) so their compute-gated waits never
                # head-of-line-block the in-order sync queue streaming the
                # input weights; the final slot uses the by-then-idle sync
                # HWDGE ring (lower launch latency).
                y_eng = nc.sync if s == order[-1] else nc.gpsimd
                if _swap_down(C):
                    B = (C + 127) // 128
                    o_slot = op.tile([128, B, H], F16, tag="o")
                    for b in range(B):
                        cb = min(128, C - b * 128)
                        for hh in range(2):
                            ps_o = ps.tile([128, 512], F32, tag="pso", bufs=2)
                            for k in range(it):
                                nc.tensor.matmul(
                                    ps_o[:cb, :],
                                    lhsT=mids[k][:, b * 128:b * 128 + cb],
                                    rhs=d_r(k, hh),
                                    start=(k == 0), stop=(k == it - 1))
                            nc.vector.tensor_copy(
                                o_slot[:cb, b, hh * 512:(hh + 1) * 512],
                                ps_o[:cb, :])
                            if s == order[-1]:
                                y_eng.dma_start(
                                    yg_ds[s][:, b, hh * 512:(hh + 1) * 512],
                                    o_slot[:, b, hh * 512:(hh + 1) * 512])
                        if s != order[-1]:
                            y_eng.dma_start(yg_ds[s][:, b], o_slot[:, b])
                else:
                    o_slot = op.tile([128, HT, C], F16, tag="o")
                    for h in range(HT):
                        ps_o = ps.tile([128, 512], F32, tag="pso", bufs=2)
                        for k in range(it):
                            nc.tensor.matmul(
                                ps_o[:, :C], lhsT=d_c(k, h), rhs=mids[k],
                                start=(k == 0), stop=(k == it - 1))
                        if s == order[-1] and h % 2:
                            # trailing slot: alternate the PSUM->SBUF casts
                            # between DVE and ACT so the final casts overlap
                            # and the last output DMA issues sooner
                            nc.scalar.activation(
                                o_slot[:, h, :], ps_o[:, :C],
                                mybir.ActivationFunctionType.Copy)
                        else:
                            nc.vector.tensor_copy(o_slot[:, h, :], ps_o[:, :C])
                        if h == HT // 2 - 1:
                            # trailing slot: first half rides the gpsimd ring
                            # so its completion receipt overlaps the second
                            # half's on the sync ring
                            (nc.gpsimd if s == order[-1] else y_eng).dma_start(
                                yg_ds[s][:, :HT // 2], o_slot[:, :HT // 2])
                        elif h == HT - 1:
                            y_eng.dma_start(yg_ds[s][:, HT // 2:],
                                            o_slot[:, HT // 2:])

    nc.compile()
    return nc


def _route(x, gate_w, expert_bias):
    """fp64 replication of the reference's grouped top-k router.

    Selection margins on this problem (min ~5e-5) are orders of magnitude above
    fp32 matmul noise, so the fp64 selection matches the fp32 reference's.
    """
    logits = x.astype(np.float64) @ gate_w.astype(np.float64).T
    scores = 1.0 / (1.0 + np.exp(-logits))
    sb = scores + expert_bias.astype(np.float64)[None, :]
    grp = sb.reshape(T, N_GROUP, E // N_GROUP)
    gs = np.sort(grp, axis=-1)[:, :, -2:].sum(-1)
    gidx = np.argsort(-gs, axis=-1, kind="stable")[:, :TOPK_GROUP]
    gmask = np.zeros((T, N_GROUP), dtype=bool)
    np.put_along_axis(gmask, gidx, True, axis=1)
    emask = np.repeat(gmask, E // N_GROUP, axis=1)
    masked = np.where(emask, sb, -np.inf)
    topk = np.argsort(-masked, axis=-1, kind="stable")[:, :TOP_K]
    w = np.take_along_axis(scores, topk, axis=1)
    w = w / w.sum(-1, keepdims=True) * ROUTE_SCALE
    return topk, w


def _pretile(w, dtype):
    """[R, F] fp32 -> [128, R/128, F] (SBUF partition-major) in dtype."""
    r, f = w.shape
    return np.ascontiguousarray(
        w.reshape(r // 128, 128, f).transpose(1, 0, 2)).astype(dtype)


def _host_fallback(x, topk, w, w_gate, w_up, w_down, sw_gate, sw_up, sw_down):
    out = np.zeros((T, H), np.float64)
    for kk in range(TOP_K):
        for e in range(E):
            sel = np.where(topk[:, kk] == e)[0]
            if sel.size == 0:
                continue
            xs = x[sel].astype(np.float64)
            g = xs @ np.asarray(w_gate[e], np.float64)
            u = xs @ np.asarray(w_up[e], np.float64)
            mid = g / (1.0 + np.exp(-g)) * u
            out[sel] += (mid @ np.asarray(w_down[e], np.float64)) * w[sel, kk][:, None]
    xs = x.astype(np.float64)
    g = xs @ np.asarray(sw_gate, np.float64)
    u = xs @ np.asarray(sw_up, np.float64)
    out += (g / (1.0 + np.exp(-g)) * u) @ np.asarray(sw_down, np.float64)
    return out.astype(np.float32)


VSPLIT = 0                 # hottest experts split into 2 half-token virtual
#                            experts: cuts sum-of-slot-caps ~10% of PE work
#                            for +1.5MB/core weight traffic. Measured: the
#                            +DMA outweighs the -PE on this ridge (53.5 vs
#                            52.3us), so splitting is off.


def _make_caps(counts):
    """Split the VSPLIT hottest experts in half, then assign the virtual
    experts to (core, slot) cells by descending load; slot capacity =
    rank-octile max. Returns (caps, assign, vex) where vex[v] = (expert,
    token lo, token hi) and assign[core, slot] = v."""
    rank = np.argsort(-counts, kind="stable")
    hot = set(rank[:VSPLIT].tolist())
    vex = []
    for e in range(E):
        n = int(counts[e])
        if e in hot:
            vex.append((e, 0, n // 2))
            vex.append((e, n // 2, n))
        else:
            vex.append((e, 0, n))
    vc = np.array([hi - lo for (_, lo, hi) in vex])
    vrank = np.argsort(-vc, kind="stable")
    epcv = len(vex) // NCORES
    assign = np.empty((NCORES, epcv), dtype=np.int64)
    caps = []
    for j in range(epcv):
        octile = vrank[j * NCORES:(j + 1) * NCORES]
        assign[:, j] = octile
        caps.append(max(32, int(-(-vc[octile].max() // 8)) * 8))
    caps.append(SBLK)
    return tuple(caps), assign, vex


def kernel(hidden_states, gate_w, expert_bias, w_gate, w_up, w_down,
           sw_gate, sw_up, sw_down):
    global last_exec_time_ns
    x = np.asarray(hidden_states, dtype=np.float32)

    topk, w = _route(x, np.asarray(gate_w), np.asarray(expert_bias))

    # dispatch plan: token list + combine weights per expert
    flat_e = topk.ravel()
    order = np.argsort(flat_e, kind="stable")
    toks = np.repeat(np.arange(T), TOP_K)[order]
    cws = w.ravel()[order]
    counts = np.bincount(flat_e, minlength=E)
    starts = np.zeros(E + 1, dtype=np.int64)
    np.cumsum(counts, out=starts[1:])
    idx_e = [toks[starts[e]:starts[e + 1]] for e in range(E)]
    cw_e = [cws[starts[e]:starts[e + 1]] for e in range(E)]

    if counts.max() > 512:
        # pathologically skewed routing would exceed the PSUM free-dim limit
        # of the compiled kernel; fall back to a host computation (never hit
        # for remotely balanced routing: expected load is T*K/E = 128)
        return _host_fallback(x, topk, w, w_gate, w_up, w_down,
                              sw_gate, sw_up, sw_down)

    # expert -> (core, slot) by descending load; slot capacity = rank-octile max
    caps, assign, vex = _make_caps(counts)
    EPCV = len(caps) - 1

    if caps not in _nc_cache:
        _nc_cache[caps] = _build(caps)
    nc = _nc_cache[caps]

    # pre-tiled operands (host-side layout = SBUF layout)
    wg32 = np.asarray(w_gate, np.float32)
    wu32 = np.asarray(w_up, np.float32)
    wd32 = np.asarray(w_down, np.float32)
    swg16 = _pretile(np.asarray(sw_gate, np.float32), np.float16)    # [128,8,ISHx2]
    swu16 = _pretile(np.asarray(sw_up, np.float32), np.float16)
    swd16 = np.asarray(sw_down, np.float32).astype(np.float16)       # [IS, H]
    # xTr[p, k, t] = x[t, 128k+p]
    xTr = np.ascontiguousarray(
        x.astype(np.float16).T.reshape(KT, 128, T).transpose(1, 0, 2))

    wq_cache = {}

    def _wq(e):
        if e not in wq_cache:
            wq_cache[e] = (
                _pretile(wg32[e] * WSC, E3M4),
                _pretile(wu32[e] * WSC, E3M4),
                _pretile(np.clip(wd32[e] * DSC, -15.5, 15.5), E3M4))
        return wq_cache[e]

    in_maps = []
    for m in range(NCORES):
        im = {}
        for j in range(EPCV):
            e, lo, hi = vex[assign[m, j]]
            n = hi - lo
            xg = np.zeros((128, KT, caps[j]), np.float16)
            xg[:, :, :n] = xTr[:, :, idx_e[e][lo:hi]]
            im[f"xg{j}"] = xg
            im[f"wg{j}"], im[f"wu{j}"], im[f"wd{j}"] = _wq(e)
        blk = m % 4
        half = m // 4
        im[f"xg{EPCV}"] = np.ascontiguousarray(xTr[:, :, blk * SBLK:(blk + 1) * SBLK])
        im["swg"] = np.ascontiguousarray(
            swg16[:, :, half * ISH:(half + 1) * ISH].reshape(128, -1))
        im["swu"] = np.ascontiguousarray(
            swu16[:, :, half * ISH:(half + 1) * ISH].reshape(128, -1))
        im["swd"] = _pretile(
            swd16[half * ISH:(half + 1) * ISH, :].astype(np.float32),
            np.float16).reshape(128, -1)
        in_maps.append(im)

    trace = os.environ.get("BASS_KERNEL_TRACE") == "1"
    run = lambda: bass_utils.run_bass_kernel_spmd(
        nc, in_maps, core_ids=list(range(NCORES)), trace=trace,
        tmpdir=os.environ.get("BASS_KERNEL_TMPDIR") or None)
    try:
        res = run()
    except ModuleNotFoundError as exc:
        # Containers without the optional NTFF profile hook module crash in
        # bass_utils when tracing is requested via env; fall back to untraced.
        if "axon_hooks" not in str(exc):
            raise
        os.environ["BASS_NEVER_TRACE"] = "1"
        res = run()
    last_exec_time_ns = res.exec_time_ns

    # combine: scatter-add weighted expert outputs + shared partials.
    # routed outputs carry the 128x fp8 weight scale -> fold 1/WSC into cw.
    out = np.zeros((T, H), np.float64)
    for m in range(NCORES):
        r = res.results[m]
        for j in range(EPCV):
            e, lo, hi = vex[assign[m, j]]
            n = hi - lo
            div = WSC * DSC
            yg = r[f"yg{j}"].astype(np.float32)
            if _swap_down(caps[j]):
                ys = yg.transpose(1, 0, 2).reshape(-1, H)[:n]
            else:
                ys = yg.transpose(2, 1, 0).reshape(-1, H)[:n]
            out[idx_e[e][lo:hi]] += ys.astype(np.float64) * (cw_e[e][lo:hi] / div)[:, None]
        blk = m % 4
        ysh = r[f"yg{EPCV}"].astype(np.float32).transpose(1, 0, 2).reshape(-1, H)
        out[blk * SBLK:(blk + 1) * SBLK] += ysh
    return out.astype(np.float32)



# revision 44
# speedup vs baseline: 1.0253x; 1.0253x over previous
"""AfmoeMoE Trainium2 kernel — expert-parallel across 8 NeuronCores.

Strategy (per sharding hint): expert-parallel. The host computes the router
(cheap: [T,E] logits + grouped top-k) to build the dispatch plan, gathers each
expert's tokens into fixed-capacity slots, and ships each core 4 experts'
weights + its gathered tokens. The device runs the heavy expert MLPs
(silu(x@wg)*(x@wu) @ wd) on gathered tokens, plus 1/8 of the shared expert
(T/4 token block x I/2 intermediate half). The host then scatter-adds the
weighted expert outputs back (the "all-to-all combine") and sums the shared
partials.

Numerics: routed gate/up weights ship as float8_e3m4 scaled by 128 (max |w|
* 128 = 13.9 < 15.5 so no clipping); the matmul runs fp8 lhsT x fp16 rhs
into fp32 PSUM, and the silu activation applies the 1/128 input scale. The
128x on the up path rides through mid (fp16, well within range) and the down
matmul, and is divided out on the host during the combine. ALL routed w_down
are also e3m4 (scale 32); the shared expert stays fp16 (e3m4 there measured
2.1-2.3e-2, over budget). End-to-end rel err 1.753e-2 vs the 2e-2 budget
(measured on HW; matches the host-side emulation to 4 digits). fp8e4m3
DoubleRow (2x PE rate) was measured numerically infeasible: it requires BOTH
operands in e4m3, and quantizing x/mid costs 4-6e-2 end-to-end.

DMA: everything is packed host-side into the exact SBUF layout so each
logical load is ONE contiguous >=0.25MB dma_start per OPERAND (wg / wu / wd
per slot, x gathered per slot — deps are tile-granular, so per-operand tiles
let each matmul group release on exactly its own transfer), issued up front
in consumption order on the sync (HWDGE) queue. Completion sems fire in ring
order ~2-3us after a transfer's last byte (receipt round-trip), pacing
~0.7-1us apart — so the opening slot's wg additionally splits into k-half
tiles and the PE warmup is sized to end right at the first sems (~11.3us).
A second DGE ring (scalar or gpsimd) measurably HURTS the opening: it either
competes for SDMA engines or sits behind ACT_TABLE_LOAD. Outputs leave via
SWDGE (gpsimd) so their compute-gated waits never head-of-line-block the
input stream; the final slot's outputs use the by-then-idle sync ring.

Compute: gate/up matmuls keep features on partitions (rhs = gathered tokens,
C-row instructions). The down-proj swaps orientation where token padding
allows (lhsT = mid block, tokens become out partitions, 512-row matmuls into
a full PSUM bank) which quarters the instruction count; slots where C is far
from a 128 multiple keep the h-tile orientation. PSUM rings: psg 4 / psu 2 /
pso 2 banks (psg 4 lets the opening slot hold all four gate groups open on
the first k-half while the second is still in flight); slot order tuned so
the first slot needs the least data and the last slot has the shortest
post-arrival tail. Slot capacities are padded to 8 (data-derived per call;
kernel compiled per capacity tuple). An 18-matmul cold warmup (~3.8us)
doubles as the HAM clock-gate trip so every real matmul runs at 2.4GHz.

Measured on this problem/seed: 50.3us HW exec (baseline session: 73.0us).
The body is on the roofline ridge: the input stream (~9.3MB/core) is ~99%
DMA-busy for 12-33us while the PE is ~99% busy 11-45us at the bf16 streaming
rate; ~13us is fixed framework preamble/epilogue (sem init + per-sem reset),
measured on an empty kernel. Virtual splitting of hot experts (5 slots,
-2.7us PE) was tried and net-regressed: +1.5MB/core weight traffic costs
more than the PE it saves.
"""
import os
import numpy as np
import ml_dtypes

import concourse.bacc as bacc
import concourse.mybir as mybir
from concourse.tile import TileContext
from concourse import bass_utils

T, H, E, IE = 1024, 1024, 32, 512
N_GROUP, TOPK_GROUP, TOP_K = 4, 2, 4
ROUTE_SCALE = 2.5
NCORES = 8
EPC = E // NCORES          # experts per core
IS = 512                   # shared-expert intermediate
ISH = IS // 2              # shared intermediate half per core
SBLK = T // 4              # shared token-block size (256)
KT = H // 128              # k tiles over H
HT = H // 128              # output h tiles
F8 = mybir.dt.float8e3
F16 = mybir.dt.float16
F32 = mybir.dt.float32
E3M4 = ml_dtypes.float8_e3m4
WSC = 128.0                # fp8 gate/up weight scale (undone in act + host)
DSC = 32.0                 # fp8 down weight scale (all routed slots)

_nc_cache = {}
last_exec_time_ns = None


def _swap_down(C):
    """Swapped down-proj (tokens as out partitions, 512-row matmuls) costs
    PE rows per 128-token block regardless of occupancy; use it only when
    block padding is small, else the h-tile orientation. (A pure-streaming
    rule — swap only when 32*C >= B*4096 — measured WORSE: the h-tile
    orientation's 4x instruction count leaves LDWEIGHTS unhidden.)"""
    return ((C + 127) // 128) * 128 - C <= 32


def _build(caps):
    """Build + compile the per-core kernel.

    caps: per-slot token capacities, slots 0..EPC-1 routed (intermediate IE,
    gate/up in fp8e3), slot EPC = shared (intermediate ISH, all fp16).
    """
    nc = bacc.Bacc("TRN2", target_bir_lowering=False, debug=False, num_devices=NCORES)

    EPCV = len(caps) - 1               # routed slots (may exceed EPC with
    #                                    virtual splitting of hot experts)
    order = [EPCV - 2] + list(range(EPCV - 2)) + [EPCV, EPCV - 1]

    xg_ds = [nc.dram_tensor(f"xg{s}", [128, KT, caps[s]], F16, kind="ExternalInput")
             for s in range(EPCV + 1)]
    wg_ds = [nc.dram_tensor(f"wg{s}", [128, KT, IE], F8, kind="ExternalInput")
             for s in range(EPCV)]
    wu_ds = [nc.dram_tensor(f"wu{s}", [128, KT, IE], F8, kind="ExternalInput")
             for s in range(EPCV)]
    wd_ds = [nc.dram_tensor(f"wd{s}", [128, IE // 128, H], F8,
                            kind="ExternalInput")
             for s in range(EPCV)]
    swg_d = nc.dram_tensor("swg", [128, KT * ISH], F16, kind="ExternalInput")
    swu_d = nc.dram_tensor("swu", [128, KT * ISH], F16, kind="ExternalInput")
    swd_d = nc.dram_tensor("swd", [128, (ISH // 128) * H], F16, kind="ExternalInput")
    yg_ds = [nc.dram_tensor(
                 f"yg{s}",
                 [128, (caps[s] + 127) // 128, H] if _swap_down(caps[s])
                 else [128, HT, caps[s]],
                 F16, kind="ExternalOutput")
             for s in range(EPCV + 1)]

    with TileContext(nc) as tc:
        with tc.tile_pool(name="xp", bufs=1) as xp, \
             tc.tile_pool(name="wp", bufs=1) as wp, \
             tc.tile_pool(name="mp", bufs=4) as mp, \
             tc.tile_pool(name="op", bufs=3) as op, \
             tc.tile_pool(name="cn", bufs=1) as cn, \
             tc.tile_pool(name="ps", bufs=3, space="PSUM") as ps:

            # PE warmup: cold dummy matmuls from ~6.6us (gpsimd memset is
            # ready ~1us before the vector engine) until the first slot's
            # opening transfer completion sems fire (~11.4us: issue + first
            # byte + bytes + ~2us receipt). >=3.4us of sustained PE busy
            # trips the HAM SHORT window so the real matmuls run at 2.4GHz.
            wtile = cn.tile([128, 384], F16)
            nc.gpsimd.memset(wtile, 0.0)
            pswarm = ps.tile([128, 256], F32, tag="pso", bufs=2)
            for r in range(18):
                nc.tensor.matmul(pswarm, lhsT=wtile[:, :128], rhs=wtile[:, 128:],
                                 start=(r == 0), stop=(r == 18 - 1))

            # ---- issue every input DMA up front, in consumption order ----
            # slot order tuned on the timeline sim: the second-lightest
            # routed slot leads (small fp8 first transfer), the lightest
            # trails (smallest post-arrival tail).
            # Dependencies are TILE-granular and each transfer costs
            # ~0.65us of serialized ring overhead (descriptor-gen at issue,
            # completion receipt before the sem). So: the first two slots
            # get per-operand tiles in consumption order (wg further split
            # into k-halves for the opening), later slots ship wg|wu|wd as
            # ONE 1.5MB fp8 blob. Everything rides the sync HWDGE ring: a
            # second ring competing for SDMA engines during the opening
            # delays the critical first transfers (measured), and the
            # scalar ring is poisoned by ACT_TABLE_LOAD at its queue head.
            xg_sbs = {}
            wg_sbs = {}
            wu_sbs = {}
            wd_sbs = {}
            KH = KT // 2
            s0 = order[0]
            for si, s in enumerate(order):
                if s == EPCV:
                    xg_sbs[s] = xp.tile([128, KT, caps[s]], F16,
                                        tag=f"xg{s}", name=f"xg{s}")
                    nc.sync.dma_start(xg_sbs[s], xg_ds[s].ap())
                    swg_sb = wp.tile([128, KT * ISH], F16, tag="swg")
                    nc.sync.dma_start(swg_sb, swg_d.ap())
                    swu_sb = wp.tile([128, KT * ISH], F16, tag="swu")
                    nc.sync.dma_start(swu_sb, swu_d.ap())
                    swd_sb = wp.tile([128, (ISH // 128) * H], F16, tag="swd")
                    nc.sync.dma_start(swd_sb, swd_d.ap())
                    continue
                xg_sbs[s] = xp.tile([128, KT, caps[s]], F16,
                                    tag=f"xg{s}", name=f"xg{s}")
                wu_sbs[s] = wp.tile([128, KT, IE], F8, tag=f"wu{s}",
                                    name=f"wu{s}")
                wd_sbs[s] = wp.tile([128, IE // 128, H], F8,
                                    tag=f"wd{s}", name=f"wdt{s}")
                nc.sync.dma_start(xg_sbs[s], xg_ds[s].ap())
                if si == 0:
                    # wg k-half tiles: the opening matmul groups
                    # release on xg + wgA alone
                    wgA = wp.tile([128, KH, IE], F8, tag="wgA", name="wgA")
                    wgB = wp.tile([128, KH, IE], F8, tag="wgB", name="wgB")
                    wg_sbs[s] = (wgA, wgB)
                    nc.sync.dma_start(wgA, wg_ds[s][:, :KH])
                    nc.sync.dma_start(wgB, wg_ds[s][:, KH:])
                else:
                    wg_sbs[s] = wp.tile([128, KT, IE], F8, tag=f"wg{s}",
                                        name=f"wg{s}")
                    nc.sync.dma_start(wg_sbs[s], wg_ds[s].ap())
                nc.sync.dma_start(wu_sbs[s], wu_ds[s].ap())
                nc.sync.dma_start(wd_sbs[s], wd_ds[s].ap())

            # ---- compute, slot by slot ----
            for s in order:
                C = caps[s]
                shared = s == EPCV
                I_s = ISH if shared else IE
                it = I_s // 128
                if shared:
                    xg_sb = xg_sbs[s]

                    def x_r(k):
                        return xg_sb[:, k, :]

                    def g_l(k, i):
                        return swg_sb[:, k * ISH + i * 128:k * ISH + (i + 1) * 128]

                    def u_l(k, i):
                        return swu_sb[:, k * ISH + i * 128:k * ISH + (i + 1) * 128]

                    def d_r(k, hh):
                        return swd_sb[:, k * H + hh * 512:k * H + hh * 512 + 512]

                    def d_c(k, h):
                        return swd_sb[:, k * H + h * 128:k * H + (h + 1) * 128]
                    act_scale = 1.0
                else:
                    xg_sb = xg_sbs[s]
                    wu_sb, wd_sb = wu_sbs[s], wd_sbs[s]

                    def x_r(k, x=xg_sb):
                        return x[:, k, :]

                    if s == s0:
                        wgAB = wg_sbs[s]

                        def g_l(k, i, w=wgAB):
                            return w[k // KH][:, k % KH, i * 128:(i + 1) * 128]
                    else:
                        wg_sb = wg_sbs[s]

                        def g_l(k, i, w=wg_sb):
                            return w[:, k, i * 128:(i + 1) * 128]

                    def u_l(k, i, w=wu_sb):
                        return w[:, k, i * 128:(i + 1) * 128]

                    def d_r(k, hh, w=wd_sb):
                        return w[:, k, hh * 512:hh * 512 + 512]

                    def d_c(k, h, w=wd_sb):
                        return w[:, k, h * 128:(h + 1) * 128]
                    act_scale = 1.0 / WSC

                def emit_gu(i):
                    ps_g = ps.tile([128, C], F32, tag="psg", bufs=4, name="ps_g")
                    for k in range(KT):
                        nc.tensor.matmul(ps_g, lhsT=g_l(k, i), rhs=x_r(k),
                                         start=(k == 0), stop=(k == KT - 1))
                    ps_u = ps.tile([128, C], F32, tag="psu", bufs=2, name="ps_u")
                    for k in range(KT):
                        nc.tensor.matmul(ps_u, lhsT=u_l(k, i), rhs=x_r(k),
                                         start=(k == 0), stop=(k == KT - 1))
                    sil = mp.tile([128, C], F16, tag=f"sil{i}", name="sil")
                    nc.scalar.activation(sil, ps_g, mybir.ActivationFunctionType.Silu,
                                         scale=act_scale)
                    mid = mp.tile([128, C], F16, tag=f"mid{i}", name="mid")
                    nc.vector.tensor_mul(mid, sil, ps_u)
                    mids.append(mid)

                mids = []
                if s == s0:
                    # opening slot: open all 4 gate groups on the A k-halves
                    # so PE has 16 matmuls of work as soon as the first two
                    # transfers land, then close them on the B halves as
                    # those arrive, then run the u groups (wu lands later)
                    ps_gs = []
                    for i in range(it):
                        ps_g = ps.tile([128, C], F32, tag="psg", bufs=4,
                                       name="ps_g")
                        for k in range(KH):
                            nc.tensor.matmul(ps_g, lhsT=g_l(k, i), rhs=x_r(k),
                                             start=(k == 0), stop=False)
                        ps_gs.append(ps_g)
                    sils = []
                    for i in range(it):
                        for k in range(KH, KT):
                            nc.tensor.matmul(ps_gs[i], lhsT=g_l(k, i), rhs=x_r(k),
                                             start=False, stop=(k == KT - 1))
                        sil = mp.tile([128, C], F16, tag=f"sil{i}", name="sil")
                        nc.scalar.activation(sil, ps_gs[i],
                                             mybir.ActivationFunctionType.Silu,
                                             scale=act_scale)
                        sils.append(sil)
                    for i in range(it):
                        ps_u = ps.tile([128, C], F32, tag="psu", bufs=2,
                                       name="ps_u")
                        for k in range(KT):
                            nc.tensor.matmul(ps_u, lhsT=u_l(k, i), rhs=x_r(k),
                                             start=(k == 0), stop=(k == KT - 1))
                        mid = mp.tile([128, C], F16, tag=f"mid{i}", name="mid")
                        nc.vector.tensor_mul(mid, sils[i], ps_u)
                        mids.append(mid)
                else:
                    for i in range(it):
                        emit_gu(i)

                # down-proj: swapped (lhsT = mid block, tokens as out
                # partitions, 512-row matmuls) when token-block padding is
                # small; else classic h-tile orientation. Outputs leave via
                # SWDGE (gpsimd) so their compute-gated waits never
                # head-of-line-block the in-order sync queue streaming the
                # input weights; the final slot uses the by-then-idle sync
                # HWDGE ring (lower launch latency).
                y_eng = nc.sync if s == order[-1] else nc.gpsimd
                if _swap_down(C):
                    B = (C + 127) // 128
                    o_slot = op.tile([128, B, H], F16, tag="o")
                    for b in range(B):
                        cb = min(128, C - b * 128)
                        for hh in range(2):
                            ps_o = ps.tile([128, 512], F32, tag="pso", bufs=2)
                            for k in range(it):
                                nc.tensor.matmul(
                                    ps_o[:cb, :],
                                    lhsT=mids[k][:, b * 128:b * 128 + cb],
                                    rhs=d_r(k, hh),
                                    start=(k == 0), stop=(k == it - 1))
                            nc.vector.tensor_copy(
                                o_slot[:cb, b, hh * 512:(hh + 1) * 512],
                                ps_o[:cb, :])
                            if s == order[-1]:
                                y_eng.dma_start(
                                    yg_ds[s][:, b, hh * 512:(hh + 1) * 512],
                                    o_slot[:, b, hh * 512:(hh + 1) * 512])
                        if s != order[-1]:
                            y_eng.dma_start(yg_ds[s][:, b], o_slot[:, b])
                else:
                    o_slot = op.tile([128, HT, C], F16, tag="o")
                    for h in range(HT):
                        ps_o = ps.tile([128, 512], F32, tag="pso", bufs=2)
                        for k in range(it):
                            nc.tensor.matmul(
                                ps_o[:, :C], lhsT=d_c(k, h), rhs=mids[k],
                                start=(k == 0), stop=(k == it - 1))
                        nc.vector.tensor_copy(o_slot[:, h, :], ps_o[:, :C])
                        if h == HT // 2 - 1:
                            y_eng.dma_start(yg_ds[s][:, :HT // 2],
                                            o_slot[:, :HT // 2])
                        elif h == HT - 1:
                            y_eng.dma_start(yg_ds[s][:, HT // 2:],
                                            o_slot[:, HT // 2:])

    nc.compile()
    return nc


def _route(x, gate_w, expert_bias):
    """fp64 replication of the reference's grouped top-k router.

    Selection margins on this problem (min ~5e-5) are orders of magnitude above
    fp32 matmul noise, so the fp64 selection matches the fp32 reference's.
    """
    logits = x.astype(np.float64) @ gate_w.astype(np.float64).T
    scores = 1.0 / (1.0 + np.exp(-logits))
    sb = scores + expert_bias.astype(np.float64)[None, :]
    grp = sb.reshape(T, N_GROUP, E // N_GROUP)
    gs = np.sort(grp, axis=-1)[:, :, -2:].sum(-1)
    gidx = np.argsort(-gs, axis=-1, kind="stable")[:, :TOPK_GROUP]
    gmask = np.zeros((T, N_GROUP), dtype=bool)
    np.put_along_axis(gmask, gidx, True, axis=1)
    emask = np.repeat(gmask, E // N_GROUP, axis=1)
    masked = np.where(emask, sb, -np.inf)
    topk = np.argsort(-masked, axis=-1, kind="stable")[:, :TOP_K]
    w = np.take_along_axis(scores, topk, axis=1)
    w = w / w.sum(-1, keepdims=True) * ROUTE_SCALE
    return topk, w


def _pretile(w, dtype):
    """[R, F] fp32 -> [128, R/128, F] (SBUF partition-major) in dtype."""
    r, f = w.shape
    return np.ascontiguousarray(
        w.reshape(r // 128, 128, f).transpose(1, 0, 2)).astype(dtype)


def _host_fallback(x, topk, w, w_gate, w_up, w_down, sw_gate, sw_up, sw_down):
    out = np.zeros((T, H), np.float64)
    for kk in range(TOP_K):
        for e in range(E):
            sel = np.where(topk[:, kk] == e)[0]
            if sel.size == 0:
                continue
            xs = x[sel].astype(np.float64)
            g = xs @ np.asarray(w_gate[e], np.float64)
            u = xs @ np.asarray(w_up[e], np.float64)
            mid = g / (1.0 + np.exp(-g)) * u
            out[sel] += (mid @ np.asarray(w_down[e], np.float64)) * w[sel, kk][:, None]
    xs = x.astype(np.float64)
    g = xs @ np.asarray(sw_gate, np.float64)
    u = xs @ np.asarray(sw_up, np.float64)
    out += (g / (1.0 + np.exp(-g)) * u) @ np.asarray(sw_down, np.float64)
    return out.astype(np.float32)


VSPLIT = 0                 # hottest experts split into 2 half-token virtual
#                            experts: cuts sum-of-slot-caps ~10% of PE work
#                            for +1.5MB/core weight traffic. Measured: the
#                            +DMA outweighs the -PE on this ridge (53.5 vs
#                            52.3us), so splitting is off.


def _make_caps(counts):
    """Split the VSPLIT hottest experts in half, then assign the virtual
    experts to (core, slot) cells by descending load; slot capacity =
    rank-octile max. Returns (caps, assign, vex) where vex[v] = (expert,
    token lo, token hi) and assign[core, slot] = v."""
    rank = np.argsort(-counts, kind="stable")
    hot = set(rank[:VSPLIT].tolist())
    vex = []
    for e in range(E):
        n = int(counts[e])
        if e in hot:
            vex.append((e, 0, n // 2))
            vex.append((e, n // 2, n))
        else:
            vex.append((e, 0, n))
    vc = np.array([hi - lo for (_, lo, hi) in vex])
    vrank = np.argsort(-vc, kind="stable")
    epcv = len(vex) // NCORES
    assign = np.empty((NCORES, epcv), dtype=np.int64)
    caps = []
    for j in range(epcv):
        octile = vrank[j * NCORES:(j + 1) * NCORES]
        assign[:, j] = octile
        caps.append(max(32, int(-(-vc[octile].max() // 8)) * 8))
    caps.append(SBLK)
    return tuple(caps), assign, vex


def kernel(hidden_states, gate_w, expert_bias, w_gate, w_up, w_down,
           sw_gate, sw_up, sw_down):
    global last_exec_time_ns
    x = np.asarray(hidden_states, dtype=np.float32)

    topk, w = _route(x, np.asarray(gate_w), np.asarray(expert_bias))

    # dispatch plan: token list + combine weights per expert
    flat_e = topk.ravel()
    order = np.argsort(flat_e, kind="stable")
    toks = np.repeat(np.arange(T), TOP_K)[order]
    cws = w.ravel()[order]
    counts = np.bincount(flat_e, minlength=E)
    starts = np.zeros(E + 1, dtype=np.int64)
    np.cumsum(counts, out=starts[1:])
    idx_e = [toks[starts[e]:starts[e + 1]] for e in range(E)]
    cw_e = [cws[starts[e]:starts[e + 1]] for e in range(E)]

    if counts.max() > 512:
        # pathologically skewed routing would exceed the PSUM free-dim limit
        # of the compiled kernel; fall back to a host computation (never hit
        # for remotely balanced routing: expected load is T*K/E = 128)
        return _host_fallback(x, topk, w, w_gate, w_up, w_down,
                              sw_gate, sw_up, sw_down)

    # expert -> (core, slot) by descending load; slot capacity = rank-octile max
    caps, assign, vex = _make_caps(counts)
    EPCV = len(caps) - 1

    if caps not in _nc_cache:
        _nc_cache[caps] = _build(caps)
    nc = _nc_cache[caps]

    # pre-tiled operands (host-side layout = SBUF layout)
    wg32 = np.asarray(w_gate, np.float32)
    wu32 = np.asarray(w_up, np.float32)
    wd32 = np.asarray(w_down, np.float32)
    swg16 = _pretile(np.asarray(sw_gate, np.float32), np.float16)    # [128,8,ISHx2]
    swu16 = _pretile(np.asarray(sw_up, np.float32), np.float16)
    swd16 = np.asarray(sw_down, np.float32).astype(np.float16)       # [IS, H]
    # xTr[p, k, t] = x[t, 128k+p]
    xTr = np.ascontiguousarray(
        x.astype(np.float16).T.reshape(KT, 128, T).transpose(1, 0, 2))

    wq_cache = {}

    def _wq(e):
        if e not in wq_cache:
            wq_cache[e] = (
                _pretile(wg32[e] * WSC, E3M4),
                _pretile(wu32[e] * WSC, E3M4),
                _pretile(np.clip(wd32[e] * DSC, -15.5, 15.5), E3M4))
        return wq_cache[e]

    in_maps = []
    for m in range(NCORES):
        im = {}
        for j in range(EPCV):
            e, lo, hi = vex[assign[m, j]]
            n = hi - lo
            xg = np.zeros((128, KT, caps[j]), np.float16)
            xg[:, :, :n] = xTr[:, :, idx_e[e][lo:hi]]
            im[f"xg{j}"] = xg
            im[f"wg{j}"], im[f"wu{j}"], im[f"wd{j}"] = _wq(e)
        blk = m % 4
        half = m // 4
        im[f"xg{EPCV}"] = np.ascontiguousarray(xTr[:, :, blk * SBLK:(blk + 1) * SBLK])
        im["swg"] = np.ascontiguousarray(
            swg16[:, :, half * ISH:(half + 1) * ISH].reshape(128, -1))
        im["swu"] = np.ascontiguousarray(
            swu16[:, :, half * ISH:(half + 1) * ISH].reshape(128, -1))
        im["swd"] = _pretile(
            swd16[half * ISH:(half + 1) * ISH, :].astype(np.float32),
            np.float16).reshape(128, -1)
        in_maps.append(im)

    trace = os.environ.get("BASS_KERNEL_TRACE") == "1"
    run = lambda: bass_utils.run_bass_kernel_spmd(
        nc, in_maps, core_ids=list(range(NCORES)), trace=trace,
        tmpdir=os.environ.get("BASS_KERNEL_TMPDIR") or None)
    try:
        res = run()
    except ModuleNotFoundError as exc:
        # Containers without the optional NTFF profile hook module crash in
        # bass_utils when tracing is requested via env; fall back to untraced.
        if "axon_hooks" not in str(exc):
            raise
        os.environ["BASS_NEVER_TRACE"] = "1"
        res = run()
    last_exec_time_ns = res.exec_time_ns

    # combine: scatter-add weighted expert outputs + shared partials.
    # routed outputs carry the 128x fp8 weight scale -> fold 1/WSC into cw.
    out = np.zeros((T, H), np.float64)
    for m in range(NCORES):
        r = res.results[m]
        for j in range(EPCV):
            e, lo, hi = vex[assign[m, j]]
            n = hi - lo
            div = WSC * DSC
            yg = r[f"yg{j}"].astype(np.float32)
            if _swap_down(caps[j]):
                ys = yg.transpose(1, 0, 2).reshape(-1, H)[:n]
            else:
                ys = yg.transpose(2, 1, 0).reshape(-1, H)[:n]
            out[idx_e[e][lo:hi]] += ys.astype(np.float64) * (cw_e[e][lo:hi] / div)[:, None]
        blk = m % 4
        ysh = r[f"yg{EPCV}"].astype(np.float32).transpose(1, 0, 2).reshape(-1, H)
        out[blk * SBLK:(blk + 1) * SBLK] += ysh
    return out.astype(np.float32)

